# revision 22
# baseline (speedup 1.0000x reference)
"""Trainium2 Bass kernel for nn_ByteEncoder (multi-scale conv stem + per-channel LRU).

Sharding: 8 cores = (batch b in 0..3) x (time-half h in 0..1). Each core runs an
identical SPMD program over raw steps [t0-512, t0+4096) (t0 = h*4096), i.e. a
128-scan-step warmup plus its 1024 output scan steps. The warmup region is
masked to zero for h=0 cores (reference scan starts at state 0) and uses real
left-context for h=1 cores (per-channel decay lambda^128 < 1e-23).

The embedding lookup is algebraically fused into the conv stem: for one-hot
inputs, conv_k(embed[x]) == sum_taps (embed @ conv_w[:,:,j])[x[t+off]], so the
stem becomes matmuls of precontracted [256-vocab x 256-ch] tables against
one-hot columns built on-chip (iota + is_equal).

v2: single fused per-tile pipeline (stem -> down-conv -> LN -> transpose ->
b-proj -> scan -> c-proj -> out LN), all matmul operands in bfloat16, h_multi
and h_down kept in SBUF (no DRAM bounce; only the small LN output z bounces
through DRAM for the later residual read). This keeps the PE continuously fed
at full p-state with no phase barriers.
"""
import numpy as np

import concourse.bass as bass
import concourse.tile as tile
from concourse import mybir, bacc
from concourse.bass_utils import run_bass_kernel_spmd
from concourse.masks import make_identity

P = 128
D = 1024
B = 4
T = 8192
VOCAB = 256
SENTINEL = 512.0  # out-of-range token -> one-hot col is all zero

W_SCAN = 128            # warmup scan steps
S_LOC = 1024 + W_SCAN   # scan steps computed per core (chunk 0 = warmup)
T_LOC = 4 * S_LOC       # raw steps per core (4608)
X_LOC = T_LOC + 8       # x slice incl conv halo (left 4, right 3, +1 pad)
N_TT = T_LOC // 512     # 9 T-tiles (each = 512 raw = 128 scan steps)
N_CH = S_LOC // 128     # 9 scan chunks
GROUPS = [(0, 4), (4, 4), (8, 1)]  # (first chunk, n chunks) for b-proj/scan

f32 = mybir.dt.float32
bf16 = mybir.dt.bfloat16
AF = mybir.ActivationFunctionType
OP = mybir.AluOpType

import os as _os
# CoreSim doesn't implement Gelu; sim_debug.py sets this to run the whole
# pipeline with Identity instead (and compares against a matching reference)
_AF_STEM = AF.Identity if _os.environ.get("BASS_SIM_IDENT") else AF.Gelu

# (conv_id, kernel_size, pad); tap offset = j - pad
CONVS = [(1, 0), (2, 1), (4, 2), (8, 4)]
TAPS = []  # (conv_id, j, off)
for ci, (K, pad) in enumerate(CONVS):
    for j in range(K):
        TAPS.append((ci, j, j - pad))
N_TAPS = len(TAPS)  # 15
TAPS_OF_CONV = [[kk for kk, (ci, _, _) in enumerate(TAPS) if ci == c] for c in range(4)]

_CACHE = {}


def _build():
    nc = bacc.Bacc()

    x_d = nc.declare_dram_parameter("x_loc", [X_LOC], bf16, isOutput=False)
    mask_d = nc.declare_dram_parameter("mask", [P], f32, isOutput=False)
    stem_d = nc.declare_dram_parameter("stem_w", [2, P, N_TAPS, 256], bf16, isOutput=False)
    # smalls = lam_ct | convb | bb2, one [P, 24] f32 DMA
    smalls_d = nc.declare_dram_parameter("smalls", [P, 24], f32, isOutput=False)
    dw_d = nc.declare_dram_parameter("down_wt", [P, 4, 8, D], bf16, isOutput=False)
    # vecs = down_b | slnw | cb2 | lruw | lrub, one broadcast DMA
    vecs_d = nc.declare_dram_parameter("vecs", [5 * D], bf16, isOutput=False)
    bw_d = nc.declare_dram_parameter("b_wt", [P, 8, D], bf16, isOutput=False)
    cw_d = nc.declare_dram_parameter("c_wt", [P, 8, D], bf16, isOutput=False)

    out_d = nc.declare_dram_parameter("out", [1024, D], f32, isOutput=True)

    with tile.TileContext(nc) as tc:
        with tc.tile_pool(name="gw", bufs=1) as gw, \
             tc.tile_pool(name="big", bufs=1) as big, \
             tc.tile_pool(name="hmp", bufs=2) as hmp, \
             tc.tile_pool(name="hstp", bufs=2) as hstp, \
             tc.tile_pool(name="tp", bufs=2) as tp, \
             tc.tile_pool(name="ps_s", bufs=2, space="PSUM") as ps_s, \
             tc.tile_pool(name="ps_d", bufs=2, space="PSUM") as ps_d, \
             tc.tile_pool(name="ps_p", bufs=3, space="PSUM") as ps_p, \
             tc.tile_pool(name="ps_t", bufs=1, space="PSUM") as ps_t:

            # ---------------- weights (startup-critical first) ----------------
            stem_sb0 = gw.tile([P, N_TAPS, 256], bf16, name="stem_sb0")
            stem_sb1 = gw.tile([P, N_TAPS, 256], bf16, name="stem_sb1")
            stem_sbs = (stem_sb0, stem_sb1)
            x_reps = {}

            def issue_xrep(tt):
                x_rep = tp.tile([P, 520], bf16, name="x_rep", bufs=2)
                nc.sync.dma_start(
                    x_rep[:],
                    x_d[tt * 512: tt * 512 + 520][None, :].to_broadcast([P, 520]))
                x_reps[tt] = x_rep

            nc.sync.dma_start(stem_sb0[:], stem_d[0])
            issue_xrep(0)
            nc.sync.dma_start(stem_sb1[:], stem_d[1])
            issue_xrep(1)
            smalls_sb = gw.tile([P, 24], f32, name="smalls_sb")
            nc.sync.dma_start(smalls_sb[:], smalls_d[:])
            lam_sb = smalls_sb[:, 0:8]
            convb_sb = smalls_sb[:, 8:16]
            bb2_sb = smalls_sb[:, 16:24]
            mask_rep = gw.tile([P, P], f32, name="mask_rep")
            nc.sync.dma_start(mask_rep[:], mask_d[:][None, :].to_broadcast([P, P]))
            dw_sb = gw.tile([P, 4, 8, D], bf16, name="dw_sb")
            for jj in range(4):
                nc.sync.dma_start(dw_sb[:, jj, :, :], dw_d[:, jj, :, :])
            vecs_sb = gw.tile([P, 5 * D], bf16, name="vecs_sb")
            nc.sync.dma_start(vecs_sb[:],
                              vecs_d[:][None, :].to_broadcast([P, 5 * D]))
            downb_rep = vecs_sb[:, 0 * D:1 * D]
            slnw_rep = vecs_sb[:, 1 * D:2 * D]
            cb2_rep = vecs_sb[:, 2 * D:3 * D]
            lruw_rep = vecs_sb[:, 3 * D:4 * D]
            lrub_rep = vecs_sb[:, 4 * D:5 * D]
            bw_sb = gw.tile([P, 8, D], bf16, name="bw_sb")
            cw_sb = gw.tile([P, 8, D], bf16, name="cw_sb")

            # ---------------- on-chip constants ----------------
            ident = gw.tile([P, P], bf16, name="ident")
            make_identity(nc, ident)
            io0 = gw.tile([P, 1], f32, name="io0")
            io1 = gw.tile([P, 1], f32, name="io1")
            nc.gpsimd.iota(io0[:], pattern=[[0, 1]], base=0, channel_multiplier=1,
                           allow_small_or_imprecise_dtypes=True)
            nc.gpsimd.iota(io1[:], pattern=[[0, 1]], base=128, channel_multiplier=1,
                           allow_small_or_imprecise_dtypes=True)
            magic_sb = gw.tile([P, 1], mybir.dt.int32, name="magic_sb")
            nc.vector.memset(magic_sb[:], 0x5f3759df)

            h_all = big.tile([P, 8, S_LOC], bf16, name="h_all")
            z_res = big.tile([P, N_CH, D], bf16, name="z_res")
            hsT_g = [None]

            def rsqrt_eps(var_ap, name):
                """1/sqrt(var_ap + 1e-5) on the DVE (no scalar-engine table).

                Magic-constant seed + 2 Newton steps; keeps the Gelu table
                resident on the scalar engine for the whole kernel.
                """
                ve = tp.tile([P, 1], f32, name=name + "_ve", bufs=2)
                nc.vector.tensor_scalar(out=ve[:], in0=var_ap, scalar1=1e-5,
                                        scalar2=None, op0=OP.add)
                yi = tp.tile([P, 1], mybir.dt.int32, name=name + "_yi", bufs=2)
                nc.vector.tensor_scalar(out=yi[:],
                                        in0=ve[:].bitcast(mybir.dt.int32),
                                        scalar1=1, scalar2=None,
                                        op0=OP.logical_shift_right)
                nc.vector.tensor_tensor(out=yi[:], in0=magic_sb[:], in1=yi[:],
                                        op=OP.subtract)
                y = yi[:].bitcast(f32)
                t = tp.tile([P, 1], f32, name=name + "_t", bufs=2)
                for _ in range(2):
                    nc.vector.tensor_tensor(out=t[:], in0=ve[:], in1=y, op=OP.mult)
                    nc.vector.tensor_tensor(out=t[:], in0=t[:], in1=y, op=OP.mult)
                    nc.vector.tensor_scalar(out=t[:], in0=t[:], scalar1=-0.5,
                                            scalar2=1.5, op0=OP.mult, op1=OP.add)
                    nc.vector.tensor_tensor(out=yi[:].bitcast(f32), in0=y,
                                            in1=t[:], op=OP.mult)
                return yi

            def do_group(g):
                g0, gn = GROUPS[g]
                W = gn * 128
                hsT = hsT_g[0]
                for dc in range(8):
                    psb = ps_p.tile([P, 512], f32, name="psb", tag="pp")
                    for ec in range(8):
                        nc.tensor.matmul(
                            psb[:, :W],
                            bw_sb[:, ec, dc * 128:(dc + 1) * 128],
                            hsT[:, ec, :W],
                            start=(ec == 0), stop=(ec == 7))
                    # bias (+warmup mask) in-place in PSUM; scan reads PSUM
                    nc.vector.tensor_scalar(out=psb[:, :W], in0=psb[:, :W],
                                            scalar1=bb2_sb[:, dc:dc + 1],
                                            scalar2=None, op0=OP.add)
                    if g0 == 0:
                        nc.vector.tensor_tensor(out=psb[:, :128], in0=psb[:, :128],
                                                in1=mask_rep[:], op=OP.mult)
                    init = (0.0 if g0 == 0
                            else h_all[:, dc, g0 * 128 - 1: g0 * 128])
                    nc.vector.tensor_tensor_scan(
                        out=h_all[:, dc, g0 * 128: g0 * 128 + W],
                        data0=lam_sb[:, dc:dc + 1].to_broadcast([P, W]),
                        data1=psb[:, :W],
                        initial=init, op0=OP.mult, op1=OP.add)

                for lc in range(gn):
                    c = g0 + lc
                    if c == 0:
                        continue
                    res = tp.tile([P, D], f32, name="res", bufs=2)
                    # res = h_s + c_b + slnb = z*slnw + cb2
                    nc.gpsimd.tensor_tensor(out=res[:], in0=z_res[:, c, :],
                                            in1=slnw_rep[:], op=OP.mult)
                    nc.gpsimd.tensor_tensor(out=res[:], in0=res[:],
                                            in1=cb2_rep[:], op=OP.add)
                    for eh in range(2):
                        psc = ps_p.tile([P, 512], f32, name="psc", tag="pp")
                        for dc in range(8):
                            nc.tensor.matmul(
                                psc[:],
                                h_all[:, dc, c * 128:(c + 1) * 128],
                                cw_sb[:, dc, eh * 512:(eh + 1) * 512],
                                start=(dc == 0), stop=(dc == 7))
                        nc.vector.tensor_tensor(
                            out=res[:, eh * 512:(eh + 1) * 512],
                            in0=psc[:],
                            in1=res[:, eh * 512:(eh + 1) * 512],
                            op=OP.add)
                    stats2 = tp.tile([P, 2, 6], f32, name="stats2", bufs=2)
                    res_g = res[:].rearrange("p (g f) -> p g f", g=2)
                    nc.vector.bn_stats(out=stats2[:, 0, :], in_=res_g[:, 0, :])
                    nc.vector.bn_stats(out=stats2[:, 1, :], in_=res_g[:, 1, :])
                    mv2 = tp.tile([P, 2], f32, name="mv2", bufs=2)
                    nc.vector.bn_aggr(out=mv2[:], in_=stats2[:])
                    rstd2 = rsqrt_eps(mv2[:, 1:2], "rstd2")
                    nc.vector.tensor_scalar(out=res[:], in0=res[:],
                                            scalar1=mv2[:, 0:1],
                                            scalar2=rstd2[:].bitcast(f32),
                                            op0=OP.subtract, op1=OP.mult)
                    nc.vector.tensor_tensor(out=res[:], in0=res[:],
                                            in1=lruw_rep[:], op=OP.mult)
                    nc.vector.tensor_tensor(out=res[:], in0=res[:],
                                            in1=lrub_rep[:], op=OP.add)
                    nc.sync.dma_start(out_d[(c - 1) * 128: c * 128, :], res[:])

            # ---------------- fused per-tile pipeline ----------------
            for tt in range(N_TT):
                if tt + 2 < N_TT:
                    issue_xrep(tt + 2)
                if tt == 2:
                    nc.sync.dma_start(bw_sb[:], bw_d[:])
                if tt == 3:
                    nc.sync.dma_start(cw_sb[:], cw_d[:])

                x_rep = x_reps.pop(tt)
                oh = tp.tile([P, 2, 520], bf16, name="oh", bufs=2)
                nc.vector.tensor_scalar(out=oh[:, 0, :], in0=x_rep[:],
                                        scalar1=io0[:], scalar2=None,
                                        op0=OP.is_equal)
                nc.vector.tensor_scalar(out=oh[:, 1, :], in0=x_rep[:],
                                        scalar1=io1[:], scalar2=None,
                                        op0=OP.is_equal)

                hm_t = hmp.tile([P, 8, 512], bf16, name="hm_t")
                for cc in range(8):
                    ci, half = cc // 2, cc % 2
                    taps = TAPS_OF_CONV[ci]
                    ps = ps_s.tile([P, 512], f32, name="ps", tag="ps")
                    n_mm = len(taps) * 2
                    i = 0
                    for vc in range(2):
                        for kk in taps:
                            off = TAPS[kk][2]
                            nc.tensor.matmul(
                                ps[:],
                                stem_sbs[vc][:, kk, half * 128:(half + 1) * 128],
                                oh[:, vc, 4 + off: 4 + off + 512],
                                start=(i == 0), stop=(i == n_mm - 1))
                            i += 1
                    nc.scalar.activation(hm_t[:, cc, :], ps[:], _AF_STEM,
                                         bias=convb_sb[:, cc:cc + 1])

                # down-conv (stride 4) for this tile's 128 scan steps
                hd_t = tp.tile([P, D], bf16, name="hd_t", bufs=1)
                for eh in range(2):
                    psd = ps_d.tile([P, 512], f32, name="psd", tag="psd")
                    i = 0
                    for dc in range(8):
                        for j in range(4):
                            nc.tensor.matmul(
                                psd[:],
                                hm_t[:, dc, j:512:4],
                                dw_sb[:, j, dc, eh * 512:(eh + 1) * 512],
                                start=(i == 0), stop=(i == 31))
                            i += 1
                    nc.vector.tensor_tensor(
                        out=hd_t[:, eh * 512:(eh + 1) * 512], in0=psd[:],
                        in1=downb_rep[:, eh * 512:(eh + 1) * 512], op=OP.add)

                # stem LN -> z (normalized, no affine; slnw folded into b_wt)
                stats = tp.tile([P, 2, 6], f32, name="stats", bufs=2)
                hd_g = hd_t[:].rearrange("p (g f) -> p g f", g=2)
                nc.vector.bn_stats(out=stats[:, 0, :], in_=hd_g[:, 0, :])
                nc.vector.bn_stats(out=stats[:, 1, :], in_=hd_g[:, 1, :])
                mv = tp.tile([P, 2], f32, name="mv", bufs=2)
                nc.vector.bn_aggr(out=mv[:], in_=stats[:])
                rstd = rsqrt_eps(mv[:, 1:2], "rstd")
                z_t = z_res[:, tt, :]
                nc.vector.tensor_scalar(out=z_t, in0=hd_t[:],
                                        scalar1=mv[:, 0:1],
                                        scalar2=rstd[:].bitcast(f32),
                                        op0=OP.subtract, op1=OP.mult)

                # transpose z into [e-ch, scan] for the b-projection
                g = 0 if tt < 4 else (1 if tt < 8 else 2)
                cig = tt - GROUPS[g][0]
                if cig == 0:
                    hsT_g[0] = hstp.tile([P, 8, 512], bf16, name="hsT")
                hsT = hsT_g[0]
                for ec in range(8):
                    pst = ps_t.tile([P, P], bf16, name="pst", tag="pst")
                    nc.tensor.transpose(pst[:], z_res[:, tt, ec * 128:(ec + 1) * 128],
                                        ident[:])
                    nc.scalar.copy(hsT[:, ec, cig * 128:(cig + 1) * 128], pst[:])

                if tt in (3, 7, 8):
                    do_group(g)

    nc.finalize()
    return nc


def _prep_host(inputs):
    import ml_dtypes
    f = np.float32
    bf = ml_dtypes.bfloat16
    embed = np.asarray(inputs["embed"], f)
    conv_ws = [np.asarray(inputs[k], f) for k in
               ("conv1_w", "conv2_w", "conv4_w", "conv8_w")]
    conv_bs = [np.asarray(inputs[k], f) for k in
               ("conv1_b", "conv2_b", "conv4_b", "conv8_b")]
    down_w = np.asarray(inputs["down_w"], f)
    log_lam = np.asarray(inputs["log_lambda_raw"], f)
    lam = (1.0 / (1.0 + np.exp(-log_lam.astype(np.float64)))).astype(f)
    b_w = np.asarray(inputs["b_w"], f)
    c_w = np.asarray(inputs["c_w"], f)

    stem_w = np.empty((2, P, N_TAPS, 256), f)
    for kk, (ci, j, _off) in enumerate(TAPS):
        fused = embed @ conv_ws[ci][:, :, j].T        # [256v, 256c]
        stem_w[:, :, kk, :] = fused.reshape(2, P, 256)
    convb = np.concatenate(conv_bs).reshape(8, P).T.copy()      # [p, cc]

    down_wt = (down_w.transpose(1, 2, 0)                        # [d, j, e]
               .reshape(8, P, 4, D).transpose(1, 2, 0, 3).copy())  # [p, j, dc, e]
    one_m = (1.0 - lam)
    slnw = np.asarray(inputs["stem_ln_w"], f)
    slnb = np.asarray(inputs["stem_ln_b"], f)
    # values[d,t] = sum_e [(1-lam_d) b_w[d,e] slnw[e]] z^T[e,t]
    #              + (1-lam_d)(b_w[d,:] @ slnb + b_b[d])
    b_wt = ((b_w.T * one_m[None, :] * slnw[:, None])            # [e, d]
            .reshape(8, P, D).transpose(1, 0, 2).copy())        # [p, ec, d]
    bb2 = (one_m * (b_w @ slnb + np.asarray(inputs["b_b"], f))
           ).reshape(8, P).T.copy()
    c_wt = c_w.T.reshape(8, P, D).transpose(1, 0, 2).copy()     # [p, dc, e]
    lam_ct = lam.reshape(8, P).T.copy()
    cb2 = np.asarray(inputs["c_b"], f) + slnb

    smalls = np.concatenate([lam_ct, convb, bb2], axis=1).astype(f)  # [P, 24]
    vecs = np.concatenate([
        np.asarray(inputs["down_b"], f), slnw, cb2,
        np.asarray(inputs["lru_ln_w"], f), np.asarray(inputs["lru_ln_b"], f),
    ]).astype(bf)                                                    # [5*D]

    shared = dict(
        stem_w=stem_w.astype(bf),
        down_wt=down_wt.astype(bf),
        b_wt=b_wt.astype(bf), c_wt=c_wt.astype(bf),
        smalls=smalls, vecs=vecs,
    )

    x = np.asarray(inputs["x"]).astype(np.int64)
    in_maps = []
    for core in range(8):
        b, h = core // 2, core % 2
        t0 = h * 4096
        idx = t0 - 516 + np.arange(X_LOC)
        valid = (idx >= 0) & (idx < T)
        x_loc = np.full((X_LOC,), SENTINEL, bf)
        x_loc[valid] = x[b, idx[valid]].astype(bf)
        mask = np.zeros((P,), f) if h == 0 else np.ones((P,), f)
        m = dict(shared)
        m["x_loc"] = x_loc
        m["mask"] = mask
        in_maps.append(m)
    return in_maps


def kernel(**inputs) -> np.ndarray:
    if "nc" not in _CACHE:
        _CACHE["nc"] = _build()
    nc = _CACHE["nc"]
    in_maps = _prep_host(inputs)
    res = run_bass_kernel_spmd(nc, in_maps, list(range(8)))
    out = np.empty((B, 2048, D), np.float32)
    for core in range(8):
        b, h = core // 2, core % 2
        out[b, h * 1024:(h + 1) * 1024, :] = res.results[core]["out"]
    return out


# revision 24
# speedup vs baseline: 1.0237x; 1.0237x over previous
"""Trainium2 Bass kernel for nn_ByteEncoder (multi-scale conv stem + per-channel LRU).

Sharding: 8 cores = (batch b in 0..3) x (time-half h in 0..1). Each core runs an
identical SPMD program over raw steps [t0-512, t0+4096) (t0 = h*4096), i.e. a
128-scan-step warmup plus its 1024 output scan steps. The warmup region is
masked to zero for h=0 cores (reference scan starts at state 0) and uses real
left-context for h=1 cores (per-channel decay lambda^128 < 1e-23).

The embedding lookup is algebraically fused into the conv stem: for one-hot
inputs, conv_k(embed[x]) == sum_taps (embed @ conv_w[:,:,j])[x[t+off]], so the
stem becomes matmuls of precontracted [256-vocab x 256-ch] tables against
one-hot columns built on-chip (iota + is_equal).

v2: single fused per-tile pipeline (stem -> down-conv -> LN -> transpose ->
b-proj -> scan -> c-proj -> out LN), all matmul operands in bfloat16, h_multi
and h_down kept in SBUF (no DRAM bounce; only the small LN output z bounces
through DRAM for the later residual read). This keeps the PE continuously fed
at full p-state with no phase barriers.
"""
import numpy as np

import concourse.bass as bass
import concourse.tile as tile
from concourse import mybir, bacc
from concourse.bass_utils import run_bass_kernel_spmd
from concourse.masks import make_identity

P = 128
D = 1024
B = 4
T = 8192
VOCAB = 256
SENTINEL = 512.0  # out-of-range token -> one-hot col is all zero

W_SCAN = 128            # warmup scan steps
S_LOC = 1024 + W_SCAN   # scan steps computed per core (chunk 0 = warmup)
T_LOC = 4 * S_LOC       # raw steps per core (4608)
X_LOC = T_LOC + 8       # x slice incl conv halo (left 4, right 3, +1 pad)
N_TT = T_LOC // 512     # 9 T-tiles (each = 512 raw = 128 scan steps)
N_CH = S_LOC // 128     # 9 scan chunks
GROUPS = [(0, 4), (4, 4), (8, 1)]  # (first chunk, n chunks) for b-proj/scan

f32 = mybir.dt.float32
bf16 = mybir.dt.bfloat16
AF = mybir.ActivationFunctionType
OP = mybir.AluOpType

import os as _os
# CoreSim doesn't implement Gelu; sim_debug.py sets this to run the whole
# pipeline with Identity instead (and compares against a matching reference)
_AF_STEM = AF.Identity if _os.environ.get("BASS_SIM_IDENT") else AF.Gelu

# (conv_id, kernel_size, pad); tap offset = j - pad
CONVS = [(1, 0), (2, 1), (4, 2), (8, 4)]
TAPS = []  # (conv_id, j, off)
for ci, (K, pad) in enumerate(CONVS):
    for j in range(K):
        TAPS.append((ci, j, j - pad))
N_TAPS = len(TAPS)  # 15
TAPS_OF_CONV = [[kk for kk, (ci, _, _) in enumerate(TAPS) if ci == c] for c in range(4)]

_CACHE = {}


def _build():
    nc = bacc.Bacc()

    x_d = nc.declare_dram_parameter("x_loc", [X_LOC], bf16, isOutput=False)
    mask_d = nc.declare_dram_parameter("mask", [P], f32, isOutput=False)
    stem_d = nc.declare_dram_parameter("stem_w", [2, P, N_TAPS, 256], bf16, isOutput=False)
    # smalls = lam_ct | convb | bb2, one [P, 24] f32 DMA
    smalls_d = nc.declare_dram_parameter("smalls", [P, 24], f32, isOutput=False)
    dw_d = nc.declare_dram_parameter("down_wt", [P, 4, 8, D], bf16, isOutput=False)
    # vecs = down_b | slnw | cb2 | lruw | lrub, one broadcast DMA
    vecs_d = nc.declare_dram_parameter("vecs", [5 * D], bf16, isOutput=False)
    bw_d = nc.declare_dram_parameter("b_wt", [P, 8, D], bf16, isOutput=False)
    cw_d = nc.declare_dram_parameter("c_wt", [P, 8, D], bf16, isOutput=False)

    out_d = nc.declare_dram_parameter("out", [1024, D], f32, isOutput=True)

    with tile.TileContext(nc) as tc:
        with tc.tile_pool(name="gw", bufs=1) as gw, \
             tc.tile_pool(name="big", bufs=1) as big, \
             tc.tile_pool(name="hmp", bufs=2) as hmp, \
             tc.tile_pool(name="hstp", bufs=2) as hstp, \
             tc.tile_pool(name="tp", bufs=2) as tp, \
             tc.tile_pool(name="ps_s", bufs=2, space="PSUM") as ps_s, \
             tc.tile_pool(name="ps_d", bufs=2, space="PSUM") as ps_d, \
             tc.tile_pool(name="ps_p", bufs=3, space="PSUM") as ps_p, \
             tc.tile_pool(name="ps_t", bufs=1, space="PSUM") as ps_t:

            # ---------------- weights (startup-critical first) ----------------
            stem_sb0 = gw.tile([P, N_TAPS, 256], bf16, name="stem_sb0")
            stem_sb1 = gw.tile([P, N_TAPS, 256], bf16, name="stem_sb1")
            stem_sbs = (stem_sb0, stem_sb1)
            x_reps = {}

            def issue_xrep(tt):
                x_rep = tp.tile([P, 520], bf16, name="x_rep", bufs=2)
                nc.sync.dma_start(
                    x_rep[:],
                    x_d[tt * 512: tt * 512 + 520][None, :].to_broadcast([P, 520]))
                x_reps[tt] = x_rep

            issue_xrep(0)
            nc.sync.dma_start(stem_sb0[:], stem_d[0])
            nc.sync.dma_start(stem_sb1[:], stem_d[1])
            issue_xrep(1)
            smalls_sb = gw.tile([P, 24], f32, name="smalls_sb")
            nc.sync.dma_start(smalls_sb[:], smalls_d[:])
            lam_sb = smalls_sb[:, 0:8]
            convb_sb = smalls_sb[:, 8:16]
            bb2_sb = smalls_sb[:, 16:24]
            mask_rep = gw.tile([P, P], f32, name="mask_rep")
            nc.sync.dma_start(mask_rep[:], mask_d[:][None, :].to_broadcast([P, P]))
            dw_sb = gw.tile([P, 4, 8, D], bf16, name="dw_sb")
            for jj in range(4):
                nc.sync.dma_start(dw_sb[:, jj, :, :], dw_d[:, jj, :, :])
            vecs_sb = gw.tile([P, 5 * D], bf16, name="vecs_sb")
            nc.sync.dma_start(vecs_sb[:],
                              vecs_d[:][None, :].to_broadcast([P, 5 * D]))
            downb_rep = vecs_sb[:, 0 * D:1 * D]
            slnw_rep = vecs_sb[:, 1 * D:2 * D]
            cb2_rep = vecs_sb[:, 2 * D:3 * D]
            lruw_rep = vecs_sb[:, 3 * D:4 * D]
            lrub_rep = vecs_sb[:, 4 * D:5 * D]
            bw_sb = gw.tile([P, 8, D], bf16, name="bw_sb")
            cw_sb = gw.tile([P, 8, D], bf16, name="cw_sb")

            # ---------------- on-chip constants ----------------
            ident = gw.tile([P, P], bf16, name="ident")
            make_identity(nc, ident)
            io0 = gw.tile([P, 1], f32, name="io0")
            io1 = gw.tile([P, 1], f32, name="io1")
            nc.gpsimd.iota(io0[:], pattern=[[0, 1]], base=0, channel_multiplier=1,
                           allow_small_or_imprecise_dtypes=True)
            nc.gpsimd.iota(io1[:], pattern=[[0, 1]], base=128, channel_multiplier=1,
                           allow_small_or_imprecise_dtypes=True)
            magic_sb = gw.tile([P, 1], mybir.dt.int32, name="magic_sb")
            nc.vector.memset(magic_sb[:], 0x5f3759df)
            # dummy activation: pull the Gelu table into the scalar engine
            # during startup DMA instead of stalling the first stem tile
            warm_sb = gw.tile([P, 1], f32, name="warm_sb")
            nc.vector.memset(warm_sb[:], 0.0)
            nc.scalar.activation(warm_sb[:], warm_sb[:], _AF_STEM)

            h_all = big.tile([P, 8, S_LOC], bf16, name="h_all")
            z_res = big.tile([P, N_CH, D], bf16, name="z_res")
            hsT_g = [None]

            def rsqrt_eps(var_ap, name):
                """1/sqrt(var_ap + 1e-5) on the DVE (no scalar-engine table).

                Magic-constant seed + 2 Newton steps; keeps the Gelu table
                resident on the scalar engine for the whole kernel.
                """
                ve = tp.tile([P, 1], f32, name=name + "_ve", bufs=2)
                nc.vector.tensor_scalar(out=ve[:], in0=var_ap, scalar1=1e-5,
                                        scalar2=None, op0=OP.add)
                yi = tp.tile([P, 1], mybir.dt.int32, name=name + "_yi", bufs=2)
                nc.vector.tensor_scalar(out=yi[:],
                                        in0=ve[:].bitcast(mybir.dt.int32),
                                        scalar1=1, scalar2=None,
                                        op0=OP.logical_shift_right)
                nc.vector.tensor_tensor(out=yi[:], in0=magic_sb[:], in1=yi[:],
                                        op=OP.subtract)
                y = yi[:].bitcast(f32)
                t = tp.tile([P, 1], f32, name=name + "_t", bufs=2)
                for _ in range(1):
                    nc.vector.tensor_tensor(out=t[:], in0=ve[:], in1=y, op=OP.mult)
                    nc.vector.tensor_tensor(out=t[:], in0=t[:], in1=y, op=OP.mult)
                    nc.vector.tensor_scalar(out=t[:], in0=t[:], scalar1=-0.5,
                                            scalar2=1.5, op0=OP.mult, op1=OP.add)
                    nc.vector.tensor_tensor(out=yi[:].bitcast(f32), in0=y,
                                            in1=t[:], op=OP.mult)
                return yi

            def do_group(g):
                g0, gn = GROUPS[g]
                W = gn * 128
                hsT = hsT_g[0]
                for dc in range(8):
                    psb = ps_p.tile([P, 512], f32, name="psb", tag="pp")
                    for ec in range(8):
                        nc.tensor.matmul(
                            psb[:, :W],
                            bw_sb[:, ec, dc * 128:(dc + 1) * 128],
                            hsT[:, ec, :W],
                            start=(ec == 0), stop=(ec == 7))
                    # bias (+warmup mask) in-place in PSUM; scan reads PSUM
                    nc.vector.tensor_scalar(out=psb[:, :W], in0=psb[:, :W],
                                            scalar1=bb2_sb[:, dc:dc + 1],
                                            scalar2=None, op0=OP.add)
                    if g0 == 0:
                        nc.vector.tensor_tensor(out=psb[:, :128], in0=psb[:, :128],
                                                in1=mask_rep[:], op=OP.mult)
                    init = (0.0 if g0 == 0
                            else h_all[:, dc, g0 * 128 - 1: g0 * 128])
                    nc.vector.tensor_tensor_scan(
                        out=h_all[:, dc, g0 * 128: g0 * 128 + W],
                        data0=lam_sb[:, dc:dc + 1].to_broadcast([P, W]),
                        data1=psb[:, :W],
                        initial=init, op0=OP.mult, op1=OP.add)

                for lc in range(gn):
                    c = g0 + lc
                    if c == 0:
                        continue
                    res = tp.tile([P, D], f32, name="res", bufs=2)
                    # res = h_s + c_b + slnb = z*slnw + cb2
                    nc.gpsimd.tensor_tensor(out=res[:], in0=z_res[:, c, :],
                                            in1=slnw_rep[:], op=OP.mult)
                    nc.gpsimd.tensor_tensor(out=res[:], in0=res[:],
                                            in1=cb2_rep[:], op=OP.add)
                    for eh in range(2):
                        psc = ps_p.tile([P, 512], f32, name="psc", tag="pp")
                        for dc in range(8):
                            nc.tensor.matmul(
                                psc[:],
                                h_all[:, dc, c * 128:(c + 1) * 128],
                                cw_sb[:, dc, eh * 512:(eh + 1) * 512],
                                start=(dc == 0), stop=(dc == 7))
                        nc.vector.tensor_tensor(
                            out=res[:, eh * 512:(eh + 1) * 512],
                            in0=psc[:],
                            in1=res[:, eh * 512:(eh + 1) * 512],
                            op=OP.add)
                    stats2 = tp.tile([P, 2, 6], f32, name="stats2", bufs=2)
                    res_g = res[:].rearrange("p (g f) -> p g f", g=2)
                    nc.vector.bn_stats(out=stats2[:, 0, :], in_=res_g[:, 0, :])
                    nc.vector.bn_stats(out=stats2[:, 1, :], in_=res_g[:, 1, :])
                    mv2 = tp.tile([P, 2], f32, name="mv2", bufs=2)
                    nc.vector.bn_aggr(out=mv2[:], in_=stats2[:])
                    rstd2 = rsqrt_eps(mv2[:, 1:2], "rstd2")
                    nc.vector.tensor_scalar(out=res[:], in0=res[:],
                                            scalar1=mv2[:, 0:1],
                                            scalar2=rstd2[:].bitcast(f32),
                                            op0=OP.subtract, op1=OP.mult)
                    nc.vector.tensor_tensor(out=res[:], in0=res[:],
                                            in1=lruw_rep[:], op=OP.mult)
                    nc.vector.tensor_tensor(out=res[:], in0=res[:],
                                            in1=lrub_rep[:], op=OP.add)
                    nc.sync.dma_start(out_d[(c - 1) * 128: c * 128, :], res[:])

            # ---------------- fused per-tile pipeline ----------------
            for tt in range(N_TT):
                if tt + 2 < N_TT:
                    issue_xrep(tt + 2)
                if tt == 2:
                    nc.sync.dma_start(bw_sb[:], bw_d[:])
                if tt == 3:
                    nc.sync.dma_start(cw_sb[:], cw_d[:])

                x_rep = x_reps.pop(tt)
                oh = tp.tile([P, 2, 520], bf16, name="oh", bufs=2)
                nc.vector.tensor_scalar(out=oh[:, 0, :], in0=x_rep[:],
                                        scalar1=io0[:], scalar2=None,
                                        op0=OP.is_equal)
                nc.vector.tensor_scalar(out=oh[:, 1, :], in0=x_rep[:],
                                        scalar1=io1[:], scalar2=None,
                                        op0=OP.is_equal)

                hm_t = hmp.tile([P, 8, 512], bf16, name="hm_t")
                for cc in range(8):
                    ci, half = cc // 2, cc % 2
                    taps = TAPS_OF_CONV[ci]
                    ps = ps_s.tile([P, 512], f32, name="ps", tag="ps")
                    n_mm = len(taps) * 2
                    i = 0
                    for vc in range(2):
                        for kk in taps:
                            off = TAPS[kk][2]
                            nc.tensor.matmul(
                                ps[:],
                                stem_sbs[vc][:, kk, half * 128:(half + 1) * 128],
                                oh[:, vc, 4 + off: 4 + off + 512],
                                start=(i == 0), stop=(i == n_mm - 1))
                            i += 1
                    nc.scalar.activation(hm_t[:, cc, :], ps[:], _AF_STEM,
                                         bias=convb_sb[:, cc:cc + 1])

                # down-conv (stride 4) for this tile's 128 scan steps;
                # biased output goes straight into the z_res slice and is
                # layer-normalized in place
                hd_t = z_res[:, tt, :]
                for eh in range(2):
                    psd = ps_d.tile([P, 512], f32, name="psd", tag="psd")
                    i = 0
                    for dc in range(8):
                        for j in range(4):
                            nc.tensor.matmul(
                                psd[:],
                                hm_t[:, dc, j:512:4],
                                dw_sb[:, j, dc, eh * 512:(eh + 1) * 512],
                                start=(i == 0), stop=(i == 31))
                            i += 1
                    nc.vector.tensor_tensor(
                        out=hd_t[:, eh * 512:(eh + 1) * 512], in0=psd[:],
                        in1=downb_rep[:, eh * 512:(eh + 1) * 512], op=OP.add)

                # stem LN -> z (normalized, no affine; slnw folded into b_wt)
                stats = tp.tile([P, 2, 6], f32, name="stats", bufs=2)
                hd_g = hd_t.rearrange("p (g f) -> p g f", g=2)
                nc.vector.bn_stats(out=stats[:, 0, :], in_=hd_g[:, 0, :])
                nc.vector.bn_stats(out=stats[:, 1, :], in_=hd_g[:, 1, :])
                mv = tp.tile([P, 2], f32, name="mv", bufs=2)
                nc.vector.bn_aggr(out=mv[:], in_=stats[:])
                rstd = rsqrt_eps(mv[:, 1:2], "rstd")
                nc.vector.tensor_scalar(out=hd_t, in0=hd_t,
                                        scalar1=mv[:, 0:1],
                                        scalar2=rstd[:].bitcast(f32),
                                        op0=OP.subtract, op1=OP.mult)

                # transpose z into [e-ch, scan] for the b-projection
                g = 0 if tt < 4 else (1 if tt < 8 else 2)
                cig = tt - GROUPS[g][0]
                if cig == 0:
                    hsT_g[0] = hstp.tile([P, 8, 512], bf16, name="hsT")
                hsT = hsT_g[0]
                for ec in range(8):
                    pst = ps_t.tile([P, P], bf16, name="pst", tag="pst")
                    nc.tensor.transpose(pst[:], z_res[:, tt, ec * 128:(ec + 1) * 128],
                                        ident[:])
                    nc.scalar.copy(hsT[:, ec, cig * 128:(cig + 1) * 128], pst[:])

                if tt in (3, 7, 8):
                    do_group(g)

    nc.finalize()
    return nc


def _prep_host(inputs):
    import ml_dtypes
    f = np.float32
    bf = ml_dtypes.bfloat16
    embed = np.asarray(inputs["embed"], f)
    conv_ws = [np.asarray(inputs[k], f) for k in
               ("conv1_w", "conv2_w", "conv4_w", "conv8_w")]
    conv_bs = [np.asarray(inputs[k], f) for k in
               ("conv1_b", "conv2_b", "conv4_b", "conv8_b")]
    down_w = np.asarray(inputs["down_w"], f)
    log_lam = np.asarray(inputs["log_lambda_raw"], f)
    lam = (1.0 / (1.0 + np.exp(-log_lam.astype(np.float64)))).astype(f)
    b_w = np.asarray(inputs["b_w"], f)
    c_w = np.asarray(inputs["c_w"], f)

    stem_w = np.empty((2, P, N_TAPS, 256), f)
    for kk, (ci, j, _off) in enumerate(TAPS):
        fused = embed @ conv_ws[ci][:, :, j].T        # [256v, 256c]
        stem_w[:, :, kk, :] = fused.reshape(2, P, 256)
    convb = np.concatenate(conv_bs).reshape(8, P).T.copy()      # [p, cc]

    down_wt = (down_w.transpose(1, 2, 0)                        # [d, j, e]
               .reshape(8, P, 4, D).transpose(1, 2, 0, 3).copy())  # [p, j, dc, e]
    one_m = (1.0 - lam)
    slnw = np.asarray(inputs["stem_ln_w"], f)
    slnb = np.asarray(inputs["stem_ln_b"], f)
    # values[d,t] = sum_e [(1-lam_d) b_w[d,e] slnw[e]] z^T[e,t]
    #              + (1-lam_d)(b_w[d,:] @ slnb + b_b[d])
    b_wt = ((b_w.T * one_m[None, :] * slnw[:, None])            # [e, d]
            .reshape(8, P, D).transpose(1, 0, 2).copy())        # [p, ec, d]
    bb2 = (one_m * (b_w @ slnb + np.asarray(inputs["b_b"], f))
           ).reshape(8, P).T.copy()
    c_wt = c_w.T.reshape(8, P, D).transpose(1, 0, 2).copy()     # [p, dc, e]
    lam_ct = lam.reshape(8, P).T.copy()
    cb2 = np.asarray(inputs["c_b"], f) + slnb

    smalls = np.concatenate([lam_ct, convb, bb2], axis=1).astype(f)  # [P, 24]
    vecs = np.concatenate([
        np.asarray(inputs["down_b"], f), slnw, cb2,
        np.asarray(inputs["lru_ln_w"], f), np.asarray(inputs["lru_ln_b"], f),
    ]).astype(bf)                                                    # [5*D]

    shared = dict(
        stem_w=stem_w.astype(bf),
        down_wt=down_wt.astype(bf),
        b_wt=b_wt.astype(bf), c_wt=c_wt.astype(bf),
        smalls=smalls, vecs=vecs,
    )

    x = np.asarray(inputs["x"]).astype(np.int64)
    in_maps = []
    for core in range(8):
        b, h = core // 2, core % 2
        t0 = h * 4096
        idx = t0 - 516 + np.arange(X_LOC)
        valid = (idx >= 0) & (idx < T)
        x_loc = np.full((X_LOC,), SENTINEL, bf)
        x_loc[valid] = x[b, idx[valid]].astype(bf)
        mask = np.zeros((P,), f) if h == 0 else np.ones((P,), f)
        m = dict(shared)
        m["x_loc"] = x_loc
        m["mask"] = mask
        in_maps.append(m)
    return in_maps


def kernel(**inputs) -> np.ndarray:
    if "nc" not in _CACHE:
        _CACHE["nc"] = _build()
    nc = _CACHE["nc"]
    in_maps = _prep_host(inputs)
    res = run_bass_kernel_spmd(nc, in_maps, list(range(8)))
    out = np.empty((B, 2048, D), np.float32)
    for core in range(8):
        b, h = core // 2, core % 2
        out[b, h * 1024:(h + 1) * 1024, :] = res.results[core]["out"]
    return out


# revision 26
# speedup vs baseline: 1.0556x; 1.0312x over previous
"""Trainium2 Bass kernel for nn_ByteEncoder (multi-scale conv stem + per-channel LRU).

Sharding: 8 cores = (batch b in 0..3) x (time-half h in 0..1). Each core runs an
identical SPMD program over raw steps [t0-128, t0+4096) (t0 = h*4096), i.e. a
32-scan-step warmup plus its 1024 output scan steps. The warmup region is
masked to zero for h=0 cores (reference scan starts at state 0) and uses real
left-context for h=1 cores (per-channel decay lambda^32 ~ 1.5e-6 << 2e-2 tol).

The embedding lookup is algebraically fused into the conv stem: for one-hot
inputs, conv_k(embed[x]) == sum_taps (embed @ conv_w[:,:,j])[x[t+off]], so the
stem becomes matmuls of precontracted [256-vocab x 256-ch] tables against
one-hot columns built on-chip (iota + is_equal).

Single fused per-tile pipeline (stem -> down-conv -> LN -> transpose ->
b-proj -> scan -> c-proj -> out LN), all matmul operands in bfloat16, all
intermediates resident in SBUF. LN rstd is computed on the DVE with the
magic-constant rsqrt (+1 Newton step) so the scalar engine's Gelu table is
never evicted. This keeps the PE continuously fed at full p-state with no
phase barriers.
"""
import numpy as np

import concourse.bass as bass
import concourse.tile as tile
from concourse import mybir, bacc
from concourse.bass_utils import run_bass_kernel_spmd
from concourse.masks import make_identity

P = 128
D = 1024
B = 4
T = 8192
VOCAB = 256
SENTINEL = 512.0  # out-of-range token -> one-hot col is all zero

W_SCAN = 32             # warmup scan steps (chunk 0)
S_LOC = 1024 + W_SCAN   # scan steps computed per core
T_LOC = 4 * S_LOC       # raw steps per core (4224)
X_LOC = T_LOC + 8       # x slice incl conv halo (left 4, right 3, +1 pad)
N_CH = 9                # chunks: [32] + [128]*8; tile tt <-> chunk tt
CH_LEN = [W_SCAN] + [128] * 8
CH_START = [0] + [W_SCAN + 128 * k for k in range(8)]   # 0,32,160,...,928
GROUPS = [(0, 4), (4, 4), (8, 1)]  # (first chunk, n chunks) for b-proj/scan

f32 = mybir.dt.float32
bf16 = mybir.dt.bfloat16
AF = mybir.ActivationFunctionType
OP = mybir.AluOpType

import os as _os
# CoreSim doesn't implement Gelu; sim_debug.py sets this to run the whole
# pipeline with Identity instead (and compares against a matching reference)
_AF_STEM = AF.Identity if _os.environ.get("BASS_SIM_IDENT") else AF.Gelu

# (conv_id, kernel_size, pad); tap offset = j - pad
CONVS = [(1, 0), (2, 1), (4, 2), (8, 4)]
TAPS = []  # (conv_id, j, off)
for ci, (K, pad) in enumerate(CONVS):
    for j in range(K):
        TAPS.append((ci, j, j - pad))
N_TAPS = len(TAPS)  # 15
TAPS_OF_CONV = [[kk for kk, (ci, _, _) in enumerate(TAPS) if ci == c] for c in range(4)]

_CACHE = {}


def _build():
    nc = bacc.Bacc()

    x_d = nc.declare_dram_parameter("x_loc", [X_LOC], bf16, isOutput=False)
    mask_d = nc.declare_dram_parameter("mask", [P], f32, isOutput=False)
    stem_d = nc.declare_dram_parameter("stem_w", [2, P, N_TAPS, 256], bf16, isOutput=False)
    # smalls = lam_ct | convb | bb2, one [P, 24] f32 DMA
    smalls_d = nc.declare_dram_parameter("smalls", [P, 24], f32, isOutput=False)
    dw_d = nc.declare_dram_parameter("down_wt", [P, 4, 8, D], bf16, isOutput=False)
    # vecs = down_b | slnw | cb2 | lruw | lrub, one broadcast DMA
    vecs_d = nc.declare_dram_parameter("vecs", [5 * D], bf16, isOutput=False)
    bw_d = nc.declare_dram_parameter("b_wt", [P, 8, D], bf16, isOutput=False)
    cw_d = nc.declare_dram_parameter("c_wt", [P, 8, D], bf16, isOutput=False)

    out_d = nc.declare_dram_parameter("out", [1024, D], f32, isOutput=True)

    with tile.TileContext(nc) as tc:
        with tc.tile_pool(name="gw", bufs=1) as gw, \
             tc.tile_pool(name="big", bufs=1) as big, \
             tc.tile_pool(name="hmp", bufs=2) as hmp, \
             tc.tile_pool(name="hstp", bufs=2) as hstp, \
             tc.tile_pool(name="tp", bufs=2) as tp, \
             tc.tile_pool(name="ps_s", bufs=2, space="PSUM") as ps_s, \
             tc.tile_pool(name="ps_d", bufs=2, space="PSUM") as ps_d, \
             tc.tile_pool(name="ps_p", bufs=3, space="PSUM") as ps_p, \
             tc.tile_pool(name="ps_t", bufs=1, space="PSUM") as ps_t:

            # ---------------- weights (startup-critical first) ----------------
            stem_sb0 = gw.tile([P, N_TAPS, 256], bf16, name="stem_sb0")
            stem_sb1 = gw.tile([P, N_TAPS, 256], bf16, name="stem_sb1")
            stem_sbs = (stem_sb0, stem_sb1)
            x_reps = {}

            def issue_xrep(tt):
                w = CH_LEN[tt] * 4 + 8
                a = 0 if tt == 0 else 128 + (tt - 1) * 512
                x_rep = tp.tile([P, 520], bf16, name="x_rep", bufs=2)
                nc.sync.dma_start(
                    x_rep[:, :w],
                    x_d[a: a + w][None, :].to_broadcast([P, w]))
                x_reps[tt] = x_rep

            issue_xrep(0)
            # per-conv pieces so conv1's chains start after ~130KB, not 2MB
            for ci in range(4):
                taps = TAPS_OF_CONV[ci]
                a, b = taps[0], taps[-1] + 1
                nc.sync.dma_start(stem_sb0[:, a:b, :], stem_d[0, :, a:b, :])
                nc.sync.dma_start(stem_sb1[:, a:b, :], stem_d[1, :, a:b, :])
            issue_xrep(1)
            smalls_sb = gw.tile([P, 24], f32, name="smalls_sb")
            nc.sync.dma_start(smalls_sb[:], smalls_d[:])
            lam_sb = smalls_sb[:, 0:8]
            convb_sb = smalls_sb[:, 8:16]
            bb2_sb = smalls_sb[:, 16:24]
            mask_rep = gw.tile([P, W_SCAN], f32, name="mask_rep")
            nc.sync.dma_start(mask_rep[:],
                              mask_d[0:W_SCAN][None, :].to_broadcast([P, W_SCAN]))
            dw_sb = gw.tile([P, 4, 8, D], bf16, name="dw_sb")
            for jj in range(4):
                nc.sync.dma_start(dw_sb[:, jj, :, :], dw_d[:, jj, :, :])
            vecs_sb = gw.tile([P, 5 * D], bf16, name="vecs_sb")
            nc.sync.dma_start(vecs_sb[:],
                              vecs_d[:][None, :].to_broadcast([P, 5 * D]))
            downb_rep = vecs_sb[:, 0 * D:1 * D]
            slnw_rep = vecs_sb[:, 1 * D:2 * D]
            cb2_rep = vecs_sb[:, 2 * D:3 * D]
            lruw_rep = vecs_sb[:, 3 * D:4 * D]
            lrub_rep = vecs_sb[:, 4 * D:5 * D]
            bw_sb = gw.tile([P, 8, D], bf16, name="bw_sb")
            cw_sb = gw.tile([P, 8, D], bf16, name="cw_sb")

            # ---------------- on-chip constants ----------------
            ident = gw.tile([P, P], bf16, name="ident")
            make_identity(nc, ident)
            io0 = gw.tile([P, 1], f32, name="io0")
            io1 = gw.tile([P, 1], f32, name="io1")
            nc.gpsimd.iota(io0[:], pattern=[[0, 1]], base=0, channel_multiplier=1,
                           allow_small_or_imprecise_dtypes=True)
            nc.gpsimd.iota(io1[:], pattern=[[0, 1]], base=128, channel_multiplier=1,
                           allow_small_or_imprecise_dtypes=True)
            magic_sb = gw.tile([P, 1], mybir.dt.int32, name="magic_sb")
            nc.vector.memset(magic_sb[:], 0x5f3759df)
            # dummy activation: pull the Gelu table into the scalar engine
            # during startup DMA instead of stalling the first stem tile
            warm_sb = gw.tile([P, 1], f32, name="warm_sb")
            nc.vector.memset(warm_sb[:], 0.0)
            nc.scalar.activation(warm_sb[:], warm_sb[:], _AF_STEM)

            h_all = big.tile([P, 8, S_LOC], bf16, name="h_all")
            z_res = big.tile([P, N_CH, D], bf16, name="z_res")
            hsT_g = [None]

            def rsqrt_eps(var_ap, name, rows):
                """1/sqrt(var_ap + 1e-5) on the DVE (no scalar-engine table).

                Magic-constant seed + 1 Newton step (max rel err ~0.2%, well
                inside the 2e-2 budget); keeps the Gelu table resident on the
                scalar engine for the whole kernel.
                """
                ve = tp.tile([P, 1], f32, name=name + "_ve", bufs=2)
                nc.vector.tensor_scalar(out=ve[:rows], in0=var_ap,
                                        scalar1=1e-5, scalar2=None, op0=OP.add)
                yi = tp.tile([P, 1], mybir.dt.int32, name=name + "_yi", bufs=2)
                nc.vector.tensor_scalar(out=yi[:rows],
                                        in0=ve[:rows].bitcast(mybir.dt.int32),
                                        scalar1=1, scalar2=None,
                                        op0=OP.logical_shift_right)
                nc.vector.tensor_tensor(out=yi[:rows], in0=magic_sb[:rows],
                                        in1=yi[:rows], op=OP.subtract)
                y = yi[:rows].bitcast(f32)
                t = tp.tile([P, 1], f32, name=name + "_t", bufs=2)
                nc.vector.tensor_tensor(out=t[:rows], in0=ve[:rows], in1=y,
                                        op=OP.mult)
                nc.vector.tensor_tensor(out=t[:rows], in0=t[:rows], in1=y,
                                        op=OP.mult)
                nc.vector.tensor_scalar(out=t[:rows], in0=t[:rows], scalar1=-0.5,
                                        scalar2=1.5, op0=OP.mult, op1=OP.add)
                nc.vector.tensor_tensor(out=yi[:rows].bitcast(f32), in0=y,
                                        in1=t[:rows], op=OP.mult)
                return yi

            def do_group(g):
                g0, gn = GROUPS[g]
                a0 = CH_START[g0]
                W = CH_START[g0 + gn - 1] + CH_LEN[g0 + gn - 1] - a0
                hsT = hsT_g[0]
                for dc in range(8):
                    psb = ps_p.tile([P, 512], f32, name="psb", tag="pp")
                    for ec in range(8):
                        nc.tensor.matmul(
                            psb[:, :W],
                            bw_sb[:, ec, dc * 128:(dc + 1) * 128],
                            hsT[:, ec, :W],
                            start=(ec == 0), stop=(ec == 7))
                    # bias (+warmup mask) in-place in PSUM; scan reads PSUM
                    nc.vector.tensor_scalar(out=psb[:, :W], in0=psb[:, :W],
                                            scalar1=bb2_sb[:, dc:dc + 1],
                                            scalar2=None, op0=OP.add)
                    if g0 == 0:
                        nc.vector.tensor_tensor(out=psb[:, :W_SCAN],
                                                in0=psb[:, :W_SCAN],
                                                in1=mask_rep[:], op=OP.mult)
                    init = (0.0 if g0 == 0 else h_all[:, dc, a0 - 1: a0])
                    nc.vector.tensor_tensor_scan(
                        out=h_all[:, dc, a0: a0 + W],
                        data0=lam_sb[:, dc:dc + 1].to_broadcast([P, W]),
                        data1=psb[:, :W],
                        initial=init, op0=OP.mult, op1=OP.add)

                for lc in range(gn):
                    c = g0 + lc
                    if c == 0:
                        continue
                    ca = CH_START[c]
                    res = tp.tile([P, D], f32, name="res", bufs=2)
                    # res = h_s + c_b + slnb = z*slnw + cb2
                    nc.gpsimd.tensor_tensor(out=res[:], in0=z_res[:, c, :],
                                            in1=slnw_rep[:], op=OP.mult)
                    nc.gpsimd.tensor_tensor(out=res[:], in0=res[:],
                                            in1=cb2_rep[:], op=OP.add)
                    for eh in range(2):
                        psc = ps_p.tile([P, 512], f32, name="psc", tag="pp")
                        for dc in range(8):
                            nc.tensor.matmul(
                                psc[:],
                                h_all[:, dc, ca: ca + 128],
                                cw_sb[:, dc, eh * 512:(eh + 1) * 512],
                                start=(dc == 0), stop=(dc == 7))
                        nc.vector.tensor_tensor(
                            out=res[:, eh * 512:(eh + 1) * 512],
                            in0=psc[:],
                            in1=res[:, eh * 512:(eh + 1) * 512],
                            op=OP.add)
                    stats2 = tp.tile([P, 2, 6], f32, name="stats2", bufs=2)
                    res_g = res[:].rearrange("p (g f) -> p g f", g=2)
                    nc.vector.bn_stats(out=stats2[:, 0, :], in_=res_g[:, 0, :])
                    nc.vector.bn_stats(out=stats2[:, 1, :], in_=res_g[:, 1, :])
                    mv2 = tp.tile([P, 2], f32, name="mv2", bufs=2)
                    nc.vector.bn_aggr(out=mv2[:], in_=stats2[:])
                    rstd2 = rsqrt_eps(mv2[:, 1:2], "rstd2", P)
                    nc.vector.tensor_scalar(out=res[:], in0=res[:],
                                            scalar1=mv2[:, 0:1],
                                            scalar2=rstd2[:].bitcast(f32),
                                            op0=OP.subtract, op1=OP.mult)
                    nc.vector.tensor_tensor(out=res[:], in0=res[:],
                                            in1=lruw_rep[:], op=OP.mult)
                    nc.vector.tensor_tensor(out=res[:], in0=res[:],
                                            in1=lrub_rep[:], op=OP.add)
                    nc.sync.dma_start(out_d[(c - 1) * 128: c * 128, :], res[:])

            # ---------------- fused per-tile pipeline ----------------
            for tt in range(N_CH):
                S = CH_LEN[tt]      # scan steps this tile (32 or 128)
                R = 4 * S           # raw steps this tile (128 or 512)
                if tt + 2 < N_CH:
                    issue_xrep(tt + 2)
                if tt == 2:
                    nc.sync.dma_start(bw_sb[:], bw_d[:])
                if tt == 3:
                    nc.sync.dma_start(cw_sb[:], cw_d[:])

                x_rep = x_reps.pop(tt)
                oh = tp.tile([P, 2, 520], bf16, name="oh", bufs=2)
                nc.vector.tensor_scalar(out=oh[:, 0, :R + 8], in0=x_rep[:, :R + 8],
                                        scalar1=io0[:], scalar2=None,
                                        op0=OP.is_equal)
                nc.vector.tensor_scalar(out=oh[:, 1, :R + 8], in0=x_rep[:, :R + 8],
                                        scalar1=io1[:], scalar2=None,
                                        op0=OP.is_equal)

                hm_t = hmp.tile([P, 8, 512], bf16, name="hm_t")
                for cc in range(8):
                    ci, half = cc // 2, cc % 2
                    taps = TAPS_OF_CONV[ci]
                    ps = ps_s.tile([P, 512], f32, name="ps", tag="ps")
                    n_mm = len(taps) * 2
                    i = 0
                    for vc in range(2):
                        for kk in taps:
                            off = TAPS[kk][2]
                            nc.tensor.matmul(
                                ps[:, :R],
                                stem_sbs[vc][:, kk, half * 128:(half + 1) * 128],
                                oh[:, vc, 4 + off: 4 + off + R],
                                start=(i == 0), stop=(i == n_mm - 1))
                            i += 1
                    nc.scalar.activation(hm_t[:, cc, :R], ps[:, :R], _AF_STEM,
                                         bias=convb_sb[:, cc:cc + 1])

                # down-conv (stride 4) for this tile's scan steps; biased
                # output goes straight into the z_res slice, LN'd in place
                hd_t = z_res[:, tt, :]
                for eh in range(2):
                    psd = ps_d.tile([P, 512], f32, name="psd", tag="psd")
                    i = 0
                    for dc in range(8):
                        for j in range(4):
                            nc.tensor.matmul(
                                psd[:S, :],
                                hm_t[:, dc, j:R:4],
                                dw_sb[:, j, dc, eh * 512:(eh + 1) * 512],
                                start=(i == 0), stop=(i == 31))
                            i += 1
                    nc.vector.tensor_tensor(
                        out=hd_t[:S, eh * 512:(eh + 1) * 512], in0=psd[:S, :],
                        in1=downb_rep[:S, eh * 512:(eh + 1) * 512], op=OP.add)

                # stem LN in place (normalized, no affine; slnw lives in b_wt)
                stats = tp.tile([P, 2, 6], f32, name="stats", bufs=2)
                hd_g = hd_t.rearrange("p (g f) -> p g f", g=2)
                nc.vector.bn_stats(out=stats[:S, 0, :], in_=hd_g[:S, 0, :])
                nc.vector.bn_stats(out=stats[:S, 1, :], in_=hd_g[:S, 1, :])
                mv = tp.tile([P, 2], f32, name="mv", bufs=2)
                nc.vector.bn_aggr(out=mv[:S], in_=stats[:S])
                rstd = rsqrt_eps(mv[:S, 1:2], "rstd", S)
                nc.vector.tensor_scalar(out=hd_t[:S], in0=hd_t[:S],
                                        scalar1=mv[:S, 0:1],
                                        scalar2=rstd[:S].bitcast(f32),
                                        op0=OP.subtract, op1=OP.mult)

                # transpose z into [e-ch, scan] for the b-projection
                g = 0 if tt < 4 else (1 if tt < 8 else 2)
                cig_a = CH_START[tt] - CH_START[GROUPS[g][0]]
                if tt == GROUPS[g][0]:
                    hsT_g[0] = hstp.tile([P, 8, 512], bf16, name="hsT")
                hsT = hsT_g[0]
                for ec in range(8):
                    pst = ps_t.tile([P, P], bf16, name="pst", tag="pst")
                    nc.tensor.transpose(pst[:, :S],
                                        z_res[:S, tt, ec * 128:(ec + 1) * 128],
                                        ident[:S, :S])
                    nc.scalar.copy(hsT[:, ec, cig_a: cig_a + S], pst[:, :S])

                if tt in (3, 7, 8):
                    do_group(g)

    nc.finalize()
    return nc


def _prep_host(inputs):
    import ml_dtypes
    f = np.float32
    bf = ml_dtypes.bfloat16
    embed = np.asarray(inputs["embed"], f)
    conv_ws = [np.asarray(inputs[k], f) for k in
               ("conv1_w", "conv2_w", "conv4_w", "conv8_w")]
    conv_bs = [np.asarray(inputs[k], f) for k in
               ("conv1_b", "conv2_b", "conv4_b", "conv8_b")]
    down_w = np.asarray(inputs["down_w"], f)
    log_lam = np.asarray(inputs["log_lambda_raw"], f)
    lam = (1.0 / (1.0 + np.exp(-log_lam.astype(np.float64)))).astype(f)
    b_w = np.asarray(inputs["b_w"], f)
    c_w = np.asarray(inputs["c_w"], f)

    stem_w = np.empty((2, P, N_TAPS, 256), f)
    for kk, (ci, j, _off) in enumerate(TAPS):
        fused = embed @ conv_ws[ci][:, :, j].T        # [256v, 256c]
        stem_w[:, :, kk, :] = fused.reshape(2, P, 256)
    convb = np.concatenate(conv_bs).reshape(8, P).T.copy()      # [p, cc]

    down_wt = (down_w.transpose(1, 2, 0)                        # [d, j, e]
               .reshape(8, P, 4, D).transpose(1, 2, 0, 3).copy())  # [p, j, dc, e]
    one_m = (1.0 - lam)
    slnw = np.asarray(inputs["stem_ln_w"], f)
    slnb = np.asarray(inputs["stem_ln_b"], f)
    # values[d,t] = sum_e [(1-lam_d) b_w[d,e] slnw[e]] z^T[e,t]
    #              + (1-lam_d)(b_w[d,:] @ slnb + b_b[d])
    b_wt = ((b_w.T * one_m[None, :] * slnw[:, None])            # [e, d]
            .reshape(8, P, D).transpose(1, 0, 2).copy())        # [p, ec, d]
    bb2 = (one_m * (b_w @ slnb + np.asarray(inputs["b_b"], f))
           ).reshape(8, P).T.copy()
    c_wt = c_w.T.reshape(8, P, D).transpose(1, 0, 2).copy()     # [p, dc, e]
    lam_ct = lam.reshape(8, P).T.copy()
    cb2 = np.asarray(inputs["c_b"], f) + slnb

    smalls = np.concatenate([lam_ct, convb, bb2], axis=1).astype(f)  # [P, 24]
    vecs = np.concatenate([
        np.asarray(inputs["down_b"], f), slnw, cb2,
        np.asarray(inputs["lru_ln_w"], f), np.asarray(inputs["lru_ln_b"], f),
    ]).astype(bf)                                                    # [5*D]

    shared = dict(
        stem_w=stem_w.astype(bf),
        down_wt=down_wt.astype(bf),
        b_wt=b_wt.astype(bf), c_wt=c_wt.astype(bf),
        smalls=smalls, vecs=vecs,
    )

    x = np.asarray(inputs["x"]).astype(np.int64)
    in_maps = []
    for core in range(8):
        b, h = core // 2, core % 2
        t0 = h * 4096
        idx = t0 - (4 * W_SCAN + 4) + np.arange(X_LOC)
        valid = (idx >= 0) & (idx < T)
        x_loc = np.full((X_LOC,), SENTINEL, bf)
        x_loc[valid] = x[b, idx[valid]].astype(bf)
        mask = np.zeros((P,), f) if h == 0 else np.ones((P,), f)
        m = dict(shared)
        m["x_loc"] = x_loc
        m["mask"] = mask
        in_maps.append(m)
    return in_maps


def kernel(**inputs) -> np.ndarray:
    if "nc" not in _CACHE:
        _CACHE["nc"] = _build()
    nc = _CACHE["nc"]
    in_maps = _prep_host(inputs)
    res = run_bass_kernel_spmd(nc, in_maps, list(range(8)))
    out = np.empty((B, 2048, D), np.float32)
    for core in range(8):
        b, h = core // 2, core % 2
        out[b, h * 1024:(h + 1) * 1024, :] = res.results[core]["out"]
    return out
